# revision 13
# baseline (speedup 1.0000x reference)
"""Trainium2 Bass kernel for nn_BridgeAttentionLayer (B=4, Tx=Tv=1024, D=1024, H=16).

Sharding: 8 cores = (batch b, query-token-half). Each core computes, for its
batch, the full K/V projections (self + cross) plus queries/attention/output
for its own 512 tokens. The host reorders tokens per core so "own" tokens are
always local positions 0:512 (attention is key-order invariant; RoPE tables
are passed per-core in matching order).

v2 (perf rewrite over the v1 baseline, 746us measured):
- Softmax normalization deferred out of the per-head-pair loop: AV outputs are
  copied unnormalized to SBUF, denominators collected into one [16,512] tile,
  one batched reciprocal_approx_fast + per-pair rank-1 broadcasts apply the
  normalization at the end.  (v1 ran a 3.3us single-lane RECIPROCAL on the
  critical path of every head pair, which kept the PE clock-gate cold.)
- exp batched to [128,1024] ACT instructions (2 key chunks per call).
- One DMA per tensor: weights/inputs laid out as [128, n*width] SBUF tiles via
  a (c p) d -> p (c d) rearrange of the DRAM source.
- LN squares + applies on DVE in bf16 (2x mode); broadcast rows go through
  the shared [128,512] proj PSUM pool and are copied once to SBUF bf16.
- cos/sin tables bf16; residual x comes from the bf16 xT tile (no fp32 xTo).
- Weight prefetch staged: attention-phase weights early (3 rotating slots),
  W_f1 + W_out during attention, W_f2 during FFN1.
"""

import numpy as np
import ml_dtypes

import concourse.bass as bass
import concourse.mybir as mybir
import concourse.tile as tile
from concourse import bacc
from concourse.bass_utils import run_bass_kernel_spmd

F32 = mybir.dt.float32
BF16 = mybir.dt.bfloat16
AF = mybir.ActivationFunctionType
ALU = mybir.AluOpType

D = 1024
H = 16
DH = 64
VW = 66           # per-head V tile width: 64 values + ones col + pad (4B align)
TQ = 512          # own query tokens per core
TK = 1024         # full sequence (keys)
NCH = 8           # D / 128
EPS = 1e-5

# packed per-partition param columns: name -> (start, n_chunks)
PARAM_COLS = {}
_off = 0
for _name, _n in [
    ("lnq_w", 8), ("lnq_nw", 8), ("lnq_b", 8),
    ("lnkv_w", 8), ("lnkv_nw", 8), ("lnkv_b", 8),
    ("lnout_w", 8), ("lnout_nw", 8), ("lnout_b", 8),
    ("lnffn_w", 8), ("lnffn_nw", 8), ("lnffn_b", 8),
    ("bq", 8), ("bk", 8), ("bcq", 8), ("bck", 8),
    ("bout", 8), ("bf2", 8), ("bf1", 32),
]:
    PARAM_COLS[_name] = (_off, _n)
    _off += _n
N_PARAM_COLS = _off

_CACHE = {}


def _build_program(trivial_ln=False):
    nc = bacc.Bacc("TRN2", target_bir_lowering=False, debug=False, num_devices=8)

    def din(name, shape, dt):
        return nc.dram_tensor(name, shape, dt, kind="ExternalInput").ap()

    dram = {
        "xT": din("xT", [D, TK], BF16),        # x[b].T, local token order
        "vT": din("vT", [D, TK], BF16),        # vggt[b].T
        "wq": din("wq", [D, D], BF16),
        "wk": din("wk", [D, D], BF16),
        "wv": din("wv", [D, D], BF16),
        "wcq": din("wcq", [D, D], BF16),
        "wck": din("wck", [D, D], BF16),
        "wcv": din("wcv", [D, D], BF16),
        "wout": din("wout", [D, D], BF16),
        "wf1": din("wf1", [D, 4 * D], BF16),
        "wf2": din("wf2", [4 * D, D], BF16),
        "params": din("params", [128, N_PARAM_COLS], F32),
        "bv_row": din("bv_row", [1, D], BF16),
        "bcv_row": din("bcv_row", [1, D], BF16),
        "cosT": din("cosT", [128, TK], BF16),  # 2-head-stacked, permuted, local order
        "sinT": din("sinT", [128, TK], BF16),
        "out": nc.dram_tensor("out", [D, TQ], F32, kind="ExternalOutput").ap(),
    }

    with tile.TileContext(nc) as tc:
        _emit(nc, tc, dram, trivial_ln)

    nc.compile()
    return nc


def _emit(nc, tc, dram, trivial_ln):
    def openpool(**kw):
        cm = tc.tile_pool(**kw)
        return cm, cm.__enter__()

    def closepool(cm):
        cm.__exit__(None, None, None)

    # ------------- long-lived left-side SBUF pools -------------
    const_cm, const = openpool(name="const", bufs=1, side="left")
    pt = const.tile([128, N_PARAM_COLS], F32)
    nc.sync.dma_start(out=pt[:], in_=dram["params"][:])

    def pcol(name, i):
        start, n = PARAM_COLS[name]
        assert i < n
        return pt[:, start + i:start + i + 1]

    ones_col_bf = const.tile([128, 1], BF16)
    nc.any.memset(ones_col_bf[:], 1.0)
    ones_row_bf = const.tile([1, 128], BF16)
    nc.any.memset(ones_row_bf[:], 1.0)
    ones_row_f = const.tile([1, 128], F32)
    nc.any.memset(ones_row_f[:], 1.0)

    tmp_cm, tmp = openpool(name="tmp", bufs=3, side="left")      # sq/rope scratch
    rows_cm, rows = openpool(name="rows", bufs=4, side="left")   # srow scratch
    rows1_cm, rows1 = openpool(name="rows1", bufs=1, side="left")  # r/mr rows
    lnsb_cm, lnsb = openpool(name="lnsb", bufs=1, side="left")   # r/m bcast bf16
    xnew_cm, xnew_p = openpool(name="xnew", bufs=1, side="left")
    attn_cm, attn_p = openpool(name="attn", bufs=1, side="left")
    xown_cm, xown_p = openpool(name="xown", bufs=1, side="left")
    qk_cm, qk_pool = openpool(name="qk", bufs=1, side="left")
    v65_cm, v65_pool = openpool(name="v65", bufs=1, side="left")

    # ------------- right-side phase pools: inputs + proj weights -------------
    inp_cm, inp = openpool(name="inp", bufs=1, side="right")
    xt = inp.tile([128, NCH * TK], BF16)
    nc.sync.dma_start(out=xt[:].rearrange("p (c t) -> p c t", t=TK),
                      in_=dram["xT"].rearrange("(c p) t -> p c t", p=128))
    vt = inp.tile([128, NCH * TK], BF16)
    nc.sync.dma_start(out=vt[:].rearrange("p (c t) -> p c t", t=TK),
                      in_=dram["vT"].rearrange("(c p) t -> p c t", p=128))
    cos_t = inp.tile([128, TK], BF16)
    nc.sync.dma_start(out=cos_t[:], in_=dram["cosT"][:])
    sin_t = inp.tile([128, TK], BF16)
    nc.sync.dma_start(out=sin_t[:], in_=dram["sinT"][:])
    bvr = inp.tile([1, D], BF16)
    nc.sync.dma_start(out=bvr[:], in_=dram["bv_row"][:])
    bcvr = inp.tile([1, D], BF16)
    nc.sync.dma_start(out=bcvr[:], in_=dram["bcv_row"][:])

    w_cm, wpool = openpool(name="w", bufs=2, side="right")

    def load_w_big(name):
        t = wpool.tile([128, NCH * D], BF16, tag="w")
        nc.sync.dma_start(
            out=t[:].rearrange("p (c d) -> p c d", d=D),
            in_=dram[name].rearrange("(c p) d -> p c d", p=128))
        return t

    # residual copy of own tokens (bf16) before LN overwrites xt
    xown = xown_p.tile([128, NCH * TQ], BF16)
    nc.vector.tensor_copy(
        xown[:].rearrange("p (c t) -> p c t", t=TQ),
        xt[:].rearrange("p (c t) -> p c t", t=TK)[:, :, 0:TQ])

    # ------------- PSUM pools for LN + projections -------------
    proj_cm, proj_ps = openpool(name="proj_ps", bufs=4, space="PSUM")
    stat_cm, stat_ps = openpool(name="ln_stat", bufs=1, space="PSUM")

    def layernorm_T(src_aps, T, wname, nwname, bname, dst_aps=None):
        """Transposed-space LN over NCH chunk APs [128, T] bf16 (in-place by
        default). Stats via ones-matmuls; rstd via Ln+Exp rows on ACT;
        broadcast through the [128,512] proj PSUM pool, copied once to SBUF
        bf16; apply on DVE in bf16 2x mode."""
        if dst_aps is None:
            dst_aps = src_aps
        nhalf = T // 512
        ps_s = [stat_ps.tile([1, 512], F32, tag=f"ps_s{h}", name=f"ps_s{h}")
                for h in range(nhalf)]
        ps_q = [stat_ps.tile([1, 512], F32, tag=f"ps_q{h}", name=f"ps_q{h}")
                for h in range(nhalf)]
        for cc in range(NCH):
            s = tmp.tile([128, T], BF16, tag="sq")
            nc.vector.tensor_mul(s[:], src_aps[cc], src_aps[cc])
            for h in range(nhalf):
                cs = slice(h * 512, (h + 1) * 512)
                nc.tensor.matmul(ps_s[h][:], ones_col_bf[:], src_aps[cc][:, cs],
                                 start=(cc == 0), stop=(cc == NCH - 1))
                nc.tensor.matmul(ps_q[h][:], ones_col_bf[:], s[:, cs],
                                 start=(cc == 0), stop=(cc == NCH - 1))
        r_row = rows1.tile([1, T], BF16, tag="r_row")
        mr_row = rows1.tile([1, T], BF16, tag="mr_row")
        for h in range(nhalf):
            cs = slice(h * 512, (h + 1) * 512)
            m = rows.tile([1, 512], BF16, tag="srow")
            nc.vector.tensor_scalar_mul(m[:], ps_s[h][:], 1.0 / D)
            msq = rows.tile([1, 512], BF16, tag="srow")
            nc.vector.tensor_mul(msq[:], m[:], m[:])
            var = rows.tile([1, 512], BF16, tag="srow")
            nc.vector.scalar_tensor_tensor(var[:], ps_q[h][:], 1.0 / D, msq[:],
                                           ALU.mult, ALU.subtract)
            nc.vector.tensor_scalar_add(var[:], var[:], EPS)
            # rstd = exp(-0.5 * ln(var+eps)): stays in the ln/exp table set
            lnv = rows.tile([1, 512], BF16, tag="srow")
            nc.scalar.activation(lnv[:], var[:], AF.Ln)
            nc.scalar.activation(r_row[:, cs], lnv[:], AF.Exp, scale=-0.5)
            nc.vector.tensor_mul(mr_row[:, cs], m[:], r_row[:, cs])
        r_sb = lnsb.tile([128, T], BF16, tag="r_sb")
        m_sb = lnsb.tile([128, T], BF16, tag="m_sb")
        for h in range(nhalf):
            cs = slice(h * 512, (h + 1) * 512)
            ps_r = proj_ps.tile([128, 512], F32, tag="proj")
            nc.tensor.matmul(ps_r[:], ones_row_bf[:], r_row[:, cs],
                             start=True, stop=True)
            nc.vector.tensor_copy(r_sb[:, cs], ps_r[:])
            ps_m = proj_ps.tile([128, 512], F32, tag="proj")
            nc.tensor.matmul(ps_m[:], ones_row_bf[:], mr_row[:, cs],
                             start=True, stop=True)
            nc.vector.tensor_copy(m_sb[:, cs], ps_m[:])
        for cc in range(NCH):
            if trivial_ln:
                t1 = tmp.tile([128, T], BF16, tag="sq")
                nc.vector.tensor_mul(t1[:], src_aps[cc], r_sb[:])
                nc.vector.scalar_tensor_tensor(dst_aps[cc], m_sb[:], -1.0,
                                               t1[:], ALU.mult, ALU.add)
            else:
                t1 = tmp.tile([128, T], BF16, tag="sq")
                nc.vector.scalar_tensor_tensor(t1[:], src_aps[cc],
                                               pcol(wname, cc), r_sb[:],
                                               ALU.mult, ALU.mult)
                t2 = tmp.tile([128, T], BF16, tag="sq")
                nc.vector.scalar_tensor_tensor(t2[:], m_sb[:], pcol(nwname, cc),
                                               t1[:], ALU.mult, ALU.add)
                nc.vector.tensor_scalar_add(dst_aps[cc], t2[:], pcol(bname, cc))
        return dst_aps

    def proj_cmajor(w_t, rhs, T, bias_name, tag):
        """Y^T[fc] = sum_cc W[cc, fc-block].T @ rhs[cc][:, :T] -> NCH bf16 [128, T]."""
        outs = []
        for fc in range(NCH):
            o = qk_pool.tile([128, T], BF16, tag=f"{tag}{fc}")
            for h in range(T // 512):
                cs = slice(h * 512, (h + 1) * 512)
                ps = proj_ps.tile([128, 512], F32, tag="proj")
                for cc in range(NCH):
                    nc.tensor.matmul(
                        ps[:], w_t[:, cc * D + fc * 128:cc * D + (fc + 1) * 128],
                        rhs[cc][:, cs], start=(cc == 0), stop=(cc == NCH - 1))
                nc.vector.tensor_scalar_add(o[:, cs], ps[:], pcol(bias_name, fc))
            outs.append(o)
        return outs

    def proj_v65(w_t, rhs, bias_row, tag):
        """Token-major V with a ones column per head: NCH bf16 [128, H*VW]."""
        outs = []
        for tcb in range(NCH):
            o = v65_pool.tile([128, H * VW], BF16, tag=f"{tag}{tcb}")
            ones_view = o[:].rearrange("p (h w) -> p h w", w=VW)[:, :, DH:DH + 1]
            nc.vector.memset(ones_view, 1.0)
            for h in range(2):
                cs = slice(h * 512, (h + 1) * 512)
                ps = proj_ps.tile([128, 512], F32, tag="proj")
                for cc in range(NCH):
                    nc.tensor.matmul(ps[:], rhs[cc][:, tcb * 128:(tcb + 1) * 128],
                                     w_t[:, cc * D + h * 512:cc * D + (h + 1) * 512],
                                     start=(cc == 0), stop=False)
                nc.tensor.matmul(ps[:], ones_row_bf[:], bias_row[:, cs],
                                 start=False, stop=True)
                dst = o[:].rearrange("p (h w) -> p h w", w=VW)[:, h * 8:(h + 1) * 8, 0:DH]
                src = ps[:].rearrange("p (h w) -> p h w", w=DH)
                nc.vector.tensor_copy(dst, src)
            outs.append(o)
        return outs

    # ------------- LN(x); self projections + RoPE; LN(v); cross projections ---
    xn = layernorm_T([xt[:, cc * TK:(cc + 1) * TK] for cc in range(NCH)],
                     TK, "lnq_w", "lnq_nw", "lnq_b")

    def rope_inplace(tiles, T):
        for fc in range(NCH):
            s = tiles[fc]
            t = tmp.tile([128, T], BF16, tag="sq")
            nc.vector.tensor_mul(t[:], s[:], cos_t[:, 0:T])
            sw = tmp.tile([128, T], BF16, tag="sq")
            for hb in range(2):
                b0 = hb * 64
                nc.vector.tensor_copy(sw[b0:b0 + 32, :], s[b0 + 32:b0 + 64, :])
                nc.vector.tensor_copy(sw[b0 + 32:b0 + 64, :], s[b0:b0 + 32, :])
            u = tmp.tile([128, T], BF16, tag="sq")
            nc.vector.tensor_mul(u[:], sw[:], sin_t[:, 0:T])
            nc.vector.tensor_add(s[:], t[:], u[:])

    wq_t = load_w_big("wq")
    wk_t = load_w_big("wk")
    wv_t = load_w_big("wv")
    qT = proj_cmajor(wq_t, xn, TQ, "bq", "qT")
    rope_inplace(qT, TQ)
    kT = proj_cmajor(wk_t, xn, TK, "bk", "kT")
    rope_inplace(kT, TK)
    v65 = proj_v65(wv_t, xn, bvr, "v65s")
    wcq_t = load_w_big("wcq")
    cqT = proj_cmajor(wcq_t, xn, TQ, "bcq", "cqT")

    vn = layernorm_T([vt[:, cc * TK:(cc + 1) * TK] for cc in range(NCH)],
                     TK, "lnkv_w", "lnkv_nw", "lnkv_b")
    wck_t = load_w_big("wck")
    ckT = proj_cmajor(wck_t, vn, TK, "bck", "ckT")
    wcv_t = load_w_big("wcv")
    cv65 = proj_v65(wcv_t, vn, bcvr, "v65c")

    closepool(stat_cm)
    closepool(proj_cm)
    closepool(w_cm)
    closepool(inp_cm)        # frees xt/vt/cos/sin

    # first half of wf1 prefetched during attention (right side)
    wf1a_cm, wf1a_p = openpool(name="wf1a_p", bufs=1, side="right")
    wf1_a = wf1a_p.tile([128, 4 * 4 * D], BF16)      # chunks cc 0-3
    nc.sync.dma_start(
        out=wf1_a[:].rearrange("p (c d) -> p c d", d=4 * D),
        in_=dram["wf1"][0:512, :].rearrange("(c p) d -> p c d", p=128))

    # ------------- attention -------------
    e_cm, e_pool = openpool(name="e", bufs=3, side="left")

    sc_cm, sc_ps = openpool(name="sc_ps", bufs=2, space="PSUM")
    avo_cm, avo_ps = openpool(name="avo_ps", bufs=2, space="PSUM")
    nrm_cm, nrm_ps = openpool(name="nrm_ps", bufs=2, space="PSUM")

    unnorm = []
    for j in range(NCH):              # head pair j: heads 2j, 2j+1
        ps_o = [avo_ps.tile([128, TQ], F32, tag="avo", name=f"avo{j}_{i}")
                for i in range(2)]
        for kp in range(8):           # pairs of key chunks
            e_tiles = []
            for i, p0 in enumerate((0, 64)):
                ps_s = sc_ps.tile([128, 1024], F32, tag="score")
                for u in range(2):
                    kc = 2 * kp + u
                    k_src = kT[j] if kc < 8 else ckT[j]
                    q_src = qT[j] if kc < 8 else cqT[j]
                    csl = slice((kc % 8) * 128, (kc % 8) * 128 + 128)
                    nc.tensor.matmul(ps_s[:, u * 512:(u + 1) * 512],
                                     k_src[p0:p0 + 64, csl], q_src[p0:p0 + 64, :],
                                     start=True, stop=True, tile_position=(p0, 0))
                e = e_pool.tile([128, 1024], BF16, tag="e")
                nc.scalar.activation(e[:], ps_s[:], AF.Exp)
                e_tiles.append(e)
            for i in range(2):
                h = 2 * j + i
                hsl = slice(h * VW, h * VW + DH + 1)
                for u in range(2):
                    kc = 2 * kp + u
                    v_src = v65[kc % 8] if kc < 8 else cv65[kc % 8]
                    nc.tensor.matmul(ps_o[i][0:DH + 1, :], v_src[:, hsl],
                                     e_tiles[i][:, u * 512:(u + 1) * 512],
                                     start=(kp == 0 and u == 0),
                                     stop=(kp == 7 and u == 1))
        un = attn_p.tile([128, TQ], BF16, tag=f"attnT{j}", name=f"attnT{j}")
        nc.vector.tensor_copy(un[0:64, :], ps_o[0][0:DH, :])
        nc.vector.tensor_copy(un[64:128, :], ps_o[1][0:DH, :])
        d0 = rows.tile([1, TQ], F32, tag="drow")
        d1 = rows.tile([1, TQ], F32, tag="drow")
        nc.vector.tensor_copy(d0[:], ps_o[0][DH:DH + 1, :])
        nc.vector.tensor_copy(d1[:], ps_o[1][DH:DH + 1, :])
        r0 = rows.tile([1, TQ], F32, tag="drow")
        r1 = rows.tile([1, TQ], F32, tag="drow")
        nc.vector.reciprocal_approx_fast(r0[:], d0[:])
        nc.vector.reciprocal_approx_fast(r1[:], d1[:])
        ps_b = nrm_ps.tile([128, TQ], F32, tag="nrm")
        nc.tensor.matmul(ps_b[0:64, :], ones_row_f[:, 0:64], r0[:],
                         start=True, stop=True, tile_position=(0, 0))
        nc.tensor.matmul(ps_b[64:128, :], ones_row_f[:, 0:64], r1[:],
                         start=True, stop=True, tile_position=(0, 64))
        nc.vector.tensor_mul(un[:], un[:], ps_b[:])
        unnorm.append(un)

    closepool(nrm_cm)
    closepool(avo_cm)
    closepool(sc_cm)
    attnT = unnorm
    closepool(e_cm)
    closepool(v65_cm)
    closepool(qk_cm)

    # wf1 second half + wout arrive during LNout/out-proj
    wf1b_cm, wf1b_p = openpool(name="wf1b_p", bufs=1, side="right")
    wf1_b = wf1b_p.tile([128, 4 * 4 * D], BF16)      # chunks cc 4-7
    wout_cm, wout_p = openpool(name="wout_p", bufs=1, side="right")
    wout_t = wout_p.tile([128, NCH * D], BF16)
    nc.sync.dma_start(out=wout_t[:].rearrange("p (c d) -> p c d", d=D),
                      in_=dram["wout"].rearrange("(c p) d -> p c d", p=128))
    nc.sync.dma_start(
        out=wf1_b[:].rearrange("p (c d) -> p c d", d=4 * D),
        in_=dram["wf1"][512:1024, :].rearrange("(c p) d -> p c d", p=128))

    # ------------- LN + out projection + residual -------------
    proj_cm, proj_ps = openpool(name="proj_ps2", bufs=6, space="PSUM")
    stat_cm, stat_ps = openpool(name="ln_stat2", bufs=1, space="PSUM")

    zT = layernorm_T([a[:] for a in attnT], TQ, "lnout_w", "lnout_nw", "lnout_b")

    xnewT = []
    for fc in range(NCH):
        ps = proj_ps.tile([128, 512], F32, tag="proj")
        for cc in range(NCH):
            nc.tensor.matmul(ps[:], wout_t[:, cc * D + fc * 128:cc * D + (fc + 1) * 128],
                             zT[cc], start=(cc == 0), stop=(cc == NCH - 1))
        xnew = xnew_p.tile([128, TQ], BF16, tag=f"xnewT{fc}")
        nc.vector.scalar_tensor_tensor(xnew[:], ps[:], pcol("bout", fc),
                                       xown[:, fc * TQ:(fc + 1) * TQ],
                                       ALU.add, ALU.add)
        xnewT.append(xnew)
    closepool(xown_cm)
    closepool(attn_cm)
    closepool(wout_cm)

    # wf2 prefetch during FFN1 (right side, above wf1)
    wf2_cm, wf2_p = openpool(name="wf2_p", bufs=1, side="right")
    wf2_t = wf2_p.tile([128, 32 * D], BF16)
    nc.sync.dma_start(out=wf2_t[:].rearrange("p (c d) -> p c d", d=D),
                      in_=dram["wf2"].rearrange("(c p) d -> p c d", p=128))

    # ------------- FFN -------------
    xn3_cm, xn3_p = openpool(name="xn3", bufs=1, side="left")
    xn3 = [xn3_p.tile([128, TQ], BF16, tag=f"xn3_{cc}", name=f"xn3_{cc}")
           for cc in range(NCH)]
    layernorm_T([x[:] for x in xnewT], TQ, "lnffn_w", "lnffn_nw", "lnffn_b",
                dst_aps=[x[:] for x in xn3])

    h1_cm, h1_p = openpool(name="h1", bufs=1, side="left")
    h1 = []
    for fc in range(32):
        ps = proj_ps.tile([128, 512], F32, tag="proj")
        for cc in range(NCH):
            w_t = wf1_a if cc < 4 else wf1_b
            col = (cc % 4) * 4 * D + fc * 128
            nc.tensor.matmul(ps[:], w_t[:, col:col + 128], xn3[cc][:],
                             start=(cc == 0), stop=(cc == NCH - 1))
        o = h1_p.tile([128, TQ], BF16, tag=f"h1_{fc}")
        nc.scalar.activation(o[:], ps[:], AF.Gelu, bias=pcol("bf1", fc))
        h1.append(o)

    fin_cm, fin_p = openpool(name="fin", bufs=2, side="left")
    for fc in range(NCH):
        ps = proj_ps.tile([128, 512], F32, tag="proj")
        for cc in range(32):
            nc.tensor.matmul(ps[:], wf2_t[:, cc * D + fc * 128:cc * D + (fc + 1) * 128],
                             h1[cc][:], start=(cc == 0), stop=(cc == 31))
        fin = fin_p.tile([128, TQ], F32, tag="fin")
        nc.vector.scalar_tensor_tensor(fin[:], ps[:], pcol("bf2", fc),
                                       xnewT[fc][:], ALU.add, ALU.add)
        nc.sync.dma_start(out=dram["out"][fc * 128:(fc + 1) * 128, :], in_=fin[:])

    # ------------- teardown (LIFO per space/side) -------------
    closepool(stat_cm)
    closepool(proj_cm)
    closepool(wf2_cm)
    closepool(wf1b_cm)
    closepool(wf1a_cm)
    closepool(fin_cm)
    closepool(h1_cm)
    closepool(xn3_cm)
    closepool(xnew_cm)
    closepool(lnsb_cm)
    closepool(rows1_cm)
    closepool(rows_cm)
    closepool(tmp_cm)
    closepool(const_cm)


def _prep_inputs(inputs):
    """Host-side sharding + weight preprocessing. Returns in_maps for 8 cores."""
    bf = ml_dtypes.bfloat16
    x = np.asarray(inputs["x"], np.float32)
    vggt = np.asarray(inputs["vggt"], np.float32)

    perm = np.concatenate([np.arange(0, DH, 2), np.arange(1, DH, 2)])
    scale = 1.0 / np.sqrt(DH)

    W_qkv = np.asarray(inputs["W_qkv"], np.float32).reshape(D, H, 3, DH)
    b_qkv = np.asarray(inputs["b_qkv"], np.float32).reshape(H, 3, DH)
    W_q = (W_qkv[:, :, 0, :][:, :, perm] * scale).reshape(D, D)
    b_q = (b_qkv[:, 0, :][:, perm] * scale).reshape(D)
    W_k = W_qkv[:, :, 1, :][:, :, perm].reshape(D, D)
    b_k = b_qkv[:, 1, :][:, perm].reshape(D)
    W_v = W_qkv[:, :, 2, :].reshape(D, D)
    b_v = b_qkv[:, 2, :].reshape(D)
    W_cq = np.asarray(inputs["W_cq"], np.float32) * scale
    b_cq = np.asarray(inputs["b_cq"], np.float32) * scale
    W_kv = np.asarray(inputs["W_kv"], np.float32).reshape(D, H, 2, DH)
    b_kv = np.asarray(inputs["b_kv"], np.float32).reshape(H, 2, DH)
    W_ck = W_kv[:, :, 0, :].reshape(D, D)
    b_ck = b_kv[:, 0, :].reshape(D)
    W_cv = W_kv[:, :, 1, :].reshape(D, D)
    b_cv = b_kv[:, 1, :].reshape(D)

    # rope tables in permuted space (64 rows), stacked x2 for 2-head tiles
    inv_freq = 1.0 / (10000.0 ** (np.arange(0, DH, 2, dtype=np.float32) / DH))
    t = np.arange(TK, dtype=np.float32)
    freqs = np.einsum("i,j->ij", t, inv_freq)
    emb = np.concatenate([freqs, freqs], axis=-1)
    cos, sin = np.cos(emb), np.sin(emb)
    cosP = np.ascontiguousarray(cos[:, perm].T).astype(np.float32)   # (64, T)
    sinP = np.empty((DH, TK), np.float32)
    sinP[0:32] = -sin[:, 0::2].T
    sinP[32:64] = +sin[:, 1::2].T

    def packcols(*vecs):
        cols = []
        for v in vecs:
            cols.append(np.asarray(v, np.float32).reshape(-1, 128).T)
        return np.ascontiguousarray(np.concatenate(cols, axis=1))

    ln = {k: np.asarray(inputs[k], np.float32) for k in
          ["ln_q_w", "ln_q_b", "ln_kv_w", "ln_kv_b", "ln_out_w", "ln_out_b",
           "ln_ffn_w", "ln_ffn_b"]}
    params = packcols(
        ln["ln_q_w"], -ln["ln_q_w"], ln["ln_q_b"],
        ln["ln_kv_w"], -ln["ln_kv_w"], ln["ln_kv_b"],
        ln["ln_out_w"], -ln["ln_out_w"], ln["ln_out_b"],
        ln["ln_ffn_w"], -ln["ln_ffn_w"], ln["ln_ffn_b"],
        b_q, b_k, b_cq, b_ck,
        np.asarray(inputs["b_out"], np.float32),
        np.asarray(inputs["b_f2"], np.float32),
        np.asarray(inputs["b_f1"], np.float32),
    )
    assert params.shape == (128, N_PARAM_COLS)

    common = {
        "wq": W_q.astype(bf), "wk": W_k.astype(bf), "wv": W_v.astype(bf),
        "wcq": W_cq.astype(bf), "wck": W_ck.astype(bf), "wcv": W_cv.astype(bf),
        "wout": np.asarray(inputs["W_out"], np.float32).astype(bf),
        "wf1": np.asarray(inputs["W_f1"], np.float32).astype(bf),
        "wf2": np.asarray(inputs["W_f2"], np.float32).astype(bf),
        "params": params,
        "bv_row": np.ascontiguousarray(b_v[None, :]).astype(bf),
        "bcv_row": np.ascontiguousarray(b_cv[None, :]).astype(bf),
    }

    in_maps = []
    for core in range(8):
        b, half = core // 2, core % 2
        if half == 0:
            order = np.arange(TK)
        else:
            order = np.concatenate([np.arange(TQ, TK), np.arange(0, TQ)])
        xl = x[b][order]
        m = dict(common)
        m["xT"] = np.ascontiguousarray(xl.T).astype(bf)
        m["vT"] = np.ascontiguousarray(vggt[b].T).astype(bf)
        ctab = cosP[:, order]
        stab = sinP[:, order]
        m["cosT"] = np.ascontiguousarray(np.concatenate([ctab, ctab], axis=0)).astype(bf)
        m["sinT"] = np.ascontiguousarray(np.concatenate([stab, stab], axis=0)).astype(bf)
        in_maps.append(m)
    return in_maps


def kernel(**inputs):
    trivial = all(np.all(np.asarray(inputs[k]) == 1.0) for k in
                  ["ln_q_w", "ln_kv_w", "ln_out_w", "ln_ffn_w"]) and \
              all(np.all(np.asarray(inputs[k]) == 0.0) for k in
                  ["ln_q_b", "ln_kv_b", "ln_out_b", "ln_ffn_b"])
    key = f"nc_{trivial}"
    if key not in _CACHE:
        _CACHE[key] = _build_program(trivial_ln=trivial)
    nc = _CACHE[key]
    in_maps = _prep_inputs(inputs)
    res = run_bass_kernel_spmd(nc, in_maps, list(range(8)),
                               **_CACHE.get("run_kwargs", {}))
    _CACHE["last_result"] = res
    outp = np.empty((4, TK, D), np.float32)
    for core in range(8):
        b, half = core // 2, core % 2
        outp[b, half * TQ:(half + 1) * TQ, :] = res.results[core]["out"].T
    return outp


# revision 15
# speedup vs baseline: 1.0957x; 1.0957x over previous
"""Trainium2 Bass kernel for nn_BridgeAttentionLayer (B=4, Tx=Tv=1024, D=1024, H=16).

Sharding: 8 cores = (batch b, query-token-half). Each core computes, for its
batch, the full K/V projections (self + cross) plus queries/attention/output
for its own 512 tokens. The host reorders tokens per core so "own" tokens are
always local positions 0:512 (attention is key-order invariant; RoPE tables
are passed per-core in matching order).

v2 (perf rewrite over the v1 baseline, 746us measured):
- Softmax normalization deferred out of the per-head-pair loop: AV outputs are
  copied unnormalized to SBUF, denominators collected into one [16,512] tile,
  one batched reciprocal_approx_fast + per-pair rank-1 broadcasts apply the
  normalization at the end.  (v1 ran a 3.3us single-lane RECIPROCAL on the
  critical path of every head pair, which kept the PE clock-gate cold.)
- exp batched to [128,1024] ACT instructions (2 key chunks per call).
- One DMA per tensor: weights/inputs laid out as [128, n*width] SBUF tiles via
  a (c p) d -> p (c d) rearrange of the DRAM source.
- LN squares + applies on DVE in bf16 (2x mode); broadcast rows go through
  the shared [128,512] proj PSUM pool and are copied once to SBUF bf16.
- cos/sin tables bf16; residual x comes from the bf16 xT tile (no fp32 xTo).
- Weight prefetch staged: attention-phase weights early (3 rotating slots),
  W_f1 + W_out during attention, W_f2 during FFN1.
"""

import numpy as np
import ml_dtypes

import concourse.bass as bass
import concourse.mybir as mybir
import concourse.tile as tile
from concourse import bacc
from concourse.bass_utils import run_bass_kernel_spmd

F32 = mybir.dt.float32
BF16 = mybir.dt.bfloat16
AF = mybir.ActivationFunctionType
ALU = mybir.AluOpType

D = 1024
H = 16
DH = 64
VW = 66           # per-head V tile width: 64 values + ones col + pad (4B align)
TQ = 512          # own query tokens per core
TK = 1024         # full sequence (keys)
NCH = 8           # D / 128
EPS = 1e-5

# packed per-partition param columns: name -> (start, n_chunks)
PARAM_COLS = {}
_off = 0
for _name, _n in [
    ("lnq_w", 8), ("lnq_nw", 8), ("lnq_b", 8),
    ("lnkv_w", 8), ("lnkv_nw", 8), ("lnkv_b", 8),
    ("lnout_w", 8), ("lnout_nw", 8), ("lnout_b", 8),
    ("lnffn_w", 8), ("lnffn_nw", 8), ("lnffn_b", 8),
    ("bq", 8), ("bk", 8), ("bcq", 8), ("bck", 8),
    ("bout", 8), ("bf2", 8), ("bf1", 32),
]:
    PARAM_COLS[_name] = (_off, _n)
    _off += _n
N_PARAM_COLS = _off

_CACHE = {}


def _build_program(trivial_ln=False):
    nc = bacc.Bacc("TRN2", target_bir_lowering=False, debug=False, num_devices=8)

    def din(name, shape, dt):
        return nc.dram_tensor(name, shape, dt, kind="ExternalInput").ap()

    dram = {
        "xT": din("xT", [D, TK], BF16),        # x[b].T, local token order
        "vT": din("vT", [D, TK], BF16),        # vggt[b].T
        "wq": din("wq", [D, D], BF16),
        "wk": din("wk", [D, D], BF16),
        "wv": din("wv", [D, D], BF16),
        "wcq": din("wcq", [D, D], BF16),
        "wck": din("wck", [D, D], BF16),
        "wcv": din("wcv", [D, D], BF16),
        "wout": din("wout", [D, D], BF16),
        "wf1": din("wf1", [D, 4 * D], BF16),
        "wf2": din("wf2", [4 * D, D], BF16),
        "params": din("params", [128, N_PARAM_COLS], F32),
        "bv_row": din("bv_row", [1, D], BF16),
        "bcv_row": din("bcv_row", [1, D], BF16),
        "cosT": din("cosT", [128, TK], BF16),  # 2-head-stacked, permuted, local order
        "sinT": din("sinT", [128, TK], BF16),
        "out": nc.dram_tensor("out", [D, TQ], F32, kind="ExternalOutput").ap(),
    }

    with tile.TileContext(nc) as tc:
        _emit(nc, tc, dram, trivial_ln)

    nc.compile()
    return nc


def _emit(nc, tc, dram, trivial_ln):
    def openpool(**kw):
        cm = tc.tile_pool(**kw)
        return cm, cm.__enter__()

    def closepool(cm):
        cm.__exit__(None, None, None)

    # ------------- long-lived left-side SBUF pools -------------
    const_cm, const = openpool(name="const", bufs=1, side="left")
    pt = const.tile([128, N_PARAM_COLS], F32)
    nc.sync.dma_start(out=pt[:], in_=dram["params"][:])

    def pcol(name, i):
        start, n = PARAM_COLS[name]
        assert i < n
        return pt[:, start + i:start + i + 1]

    ones_col_bf = const.tile([128, 1], BF16)
    nc.any.memset(ones_col_bf[:], 1.0)
    ones_row_bf = const.tile([1, 128], BF16)
    nc.any.memset(ones_row_bf[:], 1.0)
    ones_row_f = const.tile([1, 128], F32)
    nc.any.memset(ones_row_f[:], 1.0)

    tmp_cm, tmp = openpool(name="tmp", bufs=3, side="left")      # sq/rope scratch
    rows_cm, rows = openpool(name="rows", bufs=4, side="left")   # srow scratch
    rows1_cm, rows1 = openpool(name="rows1", bufs=1, side="left")  # r/mr rows
    lnsb_cm, lnsb = openpool(name="lnsb", bufs=1, side="left")   # r/m bcast bf16
    xnew_cm, xnew_p = openpool(name="xnew", bufs=1, side="left")
    attn_cm, attn_p = openpool(name="attn", bufs=1, side="left")
    xown_cm, xown_p = openpool(name="xown", bufs=1, side="left")
    qk_cm, qk_pool = openpool(name="qk", bufs=1, side="left")
    v65_cm, v65_pool = openpool(name="v65", bufs=1, side="left")

    # ------------- right-side phase pools: inputs + proj weights -------------
    inp_cm, inp = openpool(name="inp", bufs=1, side="right")
    xt = inp.tile([128, NCH * TK], BF16)
    nc.sync.dma_start(out=xt[:].rearrange("p (c t) -> p c t", t=TK),
                      in_=dram["xT"].rearrange("(c p) t -> p c t", p=128))
    vt = inp.tile([128, NCH * TK], BF16)
    nc.sync.dma_start(out=vt[:].rearrange("p (c t) -> p c t", t=TK),
                      in_=dram["vT"].rearrange("(c p) t -> p c t", p=128))

    w_cm, wpool = openpool(name="w", bufs=2, side="right")
    tabs_cm, tabs = openpool(name="tabs", bufs=1, side="right")
    cos_t = tabs.tile([128, TK], BF16)
    nc.sync.dma_start(out=cos_t[:], in_=dram["cosT"][:])
    sin_t = tabs.tile([128, TK], BF16)
    nc.sync.dma_start(out=sin_t[:], in_=dram["sinT"][:])
    bvr = tabs.tile([1, D], BF16)
    nc.sync.dma_start(out=bvr[:], in_=dram["bv_row"][:])
    bcvr = tabs.tile([1, D], BF16)
    nc.sync.dma_start(out=bcvr[:], in_=dram["bcv_row"][:])

    def load_w_big(name):
        t = wpool.tile([128, NCH * D], BF16, tag="w")
        nc.sync.dma_start(
            out=t[:].rearrange("p (c d) -> p c d", d=D),
            in_=dram[name].rearrange("(c p) d -> p c d", p=128))
        return t

    # residual copy of own tokens (bf16) before LN overwrites xt
    xown = xown_p.tile([128, NCH * TQ], BF16)
    nc.vector.tensor_copy(
        xown[:].rearrange("p (c t) -> p c t", t=TQ),
        xt[:].rearrange("p (c t) -> p c t", t=TK)[:, :, 0:TQ])

    # ------------- PSUM pools for LN + projections -------------
    proj_cm, proj_ps = openpool(name="proj_ps", bufs=2, space="PSUM")
    stat_cm, stat_ps = openpool(name="ln_stat", bufs=1, space="PSUM")

    def layernorm_T(src_aps, T, wname, nwname, bname, dst_aps=None):
        """Transposed-space LN over NCH chunk APs [128, T] bf16 (in-place by
        default). Stats via ones-matmuls; rstd via Ln+Exp rows on ACT;
        broadcast through the [128,512] proj PSUM pool, copied once to SBUF
        bf16; apply on DVE in bf16 2x mode."""
        if dst_aps is None:
            dst_aps = src_aps
        nhalf = T // 512
        ps_s = [stat_ps.tile([1, 512], F32, tag=f"ps_s{h}", name=f"ps_s{h}")
                for h in range(nhalf)]
        ps_q = [stat_ps.tile([1, 512], F32, tag=f"ps_q{h}", name=f"ps_q{h}")
                for h in range(nhalf)]
        for cc in range(NCH):
            s = tmp.tile([128, T], BF16, tag="sq")
            nc.vector.tensor_mul(s[:], src_aps[cc], src_aps[cc])
            for h in range(nhalf):
                cs = slice(h * 512, (h + 1) * 512)
                nc.tensor.matmul(ps_s[h][:], ones_col_bf[:], src_aps[cc][:, cs],
                                 start=(cc == 0), stop=(cc == NCH - 1))
                nc.tensor.matmul(ps_q[h][:], ones_col_bf[:], s[:, cs],
                                 start=(cc == 0), stop=(cc == NCH - 1))
        r_row = rows1.tile([1, T], BF16, tag="r_row")
        mr_row = rows1.tile([1, T], BF16, tag="mr_row")
        for h in range(nhalf):
            cs = slice(h * 512, (h + 1) * 512)
            m = rows.tile([1, 512], BF16, tag="srow")
            nc.vector.tensor_scalar_mul(m[:], ps_s[h][:], 1.0 / D)
            msq = rows.tile([1, 512], BF16, tag="srow")
            nc.vector.tensor_mul(msq[:], m[:], m[:])
            var = rows.tile([1, 512], BF16, tag="srow")
            nc.vector.scalar_tensor_tensor(var[:], ps_q[h][:], 1.0 / D, msq[:],
                                           ALU.mult, ALU.subtract)
            nc.vector.tensor_scalar_add(var[:], var[:], EPS)
            # rstd = exp(-0.5 * ln(var+eps)): stays in the ln/exp table set
            lnv = rows.tile([1, 512], BF16, tag="srow")
            nc.scalar.activation(lnv[:], var[:], AF.Ln)
            nc.scalar.activation(r_row[:, cs], lnv[:], AF.Exp, scale=-0.5)
            nc.vector.tensor_mul(mr_row[:, cs], m[:], r_row[:, cs])
        r_sb = lnsb.tile([128, T], BF16, tag="r_sb")
        m_sb = lnsb.tile([128, T], BF16, tag="m_sb")
        for h in range(nhalf):
            cs = slice(h * 512, (h + 1) * 512)
            ps_r = proj_ps.tile([128, 512], F32, tag="proj")
            nc.tensor.matmul(ps_r[:], ones_row_bf[:], r_row[:, cs],
                             start=True, stop=True)
            nc.vector.tensor_copy(r_sb[:, cs], ps_r[:])
            ps_m = proj_ps.tile([128, 512], F32, tag="proj")
            nc.tensor.matmul(ps_m[:], ones_row_bf[:], mr_row[:, cs],
                             start=True, stop=True)
            nc.vector.tensor_copy(m_sb[:, cs], ps_m[:])
        for cc in range(NCH):
            if trivial_ln:
                t1 = tmp.tile([128, T], BF16, tag="sq")
                nc.vector.tensor_mul(t1[:], src_aps[cc], r_sb[:])
                nc.vector.scalar_tensor_tensor(dst_aps[cc], m_sb[:], -1.0,
                                               t1[:], ALU.mult, ALU.add)
            else:
                t1 = tmp.tile([128, T], BF16, tag="sq")
                nc.vector.scalar_tensor_tensor(t1[:], src_aps[cc],
                                               pcol(wname, cc), r_sb[:],
                                               ALU.mult, ALU.mult)
                t2 = tmp.tile([128, T], BF16, tag="sq")
                nc.vector.scalar_tensor_tensor(t2[:], m_sb[:], pcol(nwname, cc),
                                               t1[:], ALU.mult, ALU.add)
                nc.vector.tensor_scalar_add(dst_aps[cc], t2[:], pcol(bname, cc))
        return dst_aps

    def proj_cmajor(w_t, rhs, T, bias_name, tag):
        """Y^T[fc] = sum_cc W[cc, fc-block].T @ rhs[cc][:, :T] -> NCH bf16 [128, T]."""
        outs = []
        for fc in range(NCH):
            o = qk_pool.tile([128, T], BF16, tag=f"{tag}{fc}")
            for h in range(T // 512):
                cs = slice(h * 512, (h + 1) * 512)
                ps = proj_ps.tile([128, 512], F32, tag="proj")
                for cc in range(NCH):
                    nc.tensor.matmul(
                        ps[:], w_t[:, cc * D + fc * 128:cc * D + (fc + 1) * 128],
                        rhs[cc][:, cs], start=(cc == 0), stop=(cc == NCH - 1))
                nc.vector.tensor_scalar_add(o[:, cs], ps[:], pcol(bias_name, fc))
            outs.append(o)
        return outs

    def proj_v65(w_t, rhs, bias_row, tag):
        """Token-major V with a ones column per head: NCH bf16 [128, H*VW]."""
        outs = []
        for tcb in range(NCH):
            o = v65_pool.tile([128, H * VW], BF16, tag=f"{tag}{tcb}")
            ones_view = o[:].rearrange("p (h w) -> p h w", w=VW)[:, :, DH:DH + 1]
            nc.vector.memset(ones_view, 1.0)
            for h in range(2):
                cs = slice(h * 512, (h + 1) * 512)
                ps = proj_ps.tile([128, 512], F32, tag="proj")
                for cc in range(NCH):
                    nc.tensor.matmul(ps[:], rhs[cc][:, tcb * 128:(tcb + 1) * 128],
                                     w_t[:, cc * D + h * 512:cc * D + (h + 1) * 512],
                                     start=(cc == 0), stop=False)
                nc.tensor.matmul(ps[:], ones_row_bf[:], bias_row[:, cs],
                                 start=False, stop=True)
                dst = o[:].rearrange("p (h w) -> p h w", w=VW)[:, h * 8:(h + 1) * 8, 0:DH]
                src = ps[:].rearrange("p (h w) -> p h w", w=DH)
                nc.vector.tensor_copy(dst, src)
            outs.append(o)
        return outs

    # ------------- LN(x); self projections + RoPE; LN(v); cross projections ---
    xn = layernorm_T([xt[:, cc * TK:(cc + 1) * TK] for cc in range(NCH)],
                     TK, "lnq_w", "lnq_nw", "lnq_b")

    def rope_inplace(tiles, T):
        for fc in range(NCH):
            s = tiles[fc]
            t = tmp.tile([128, T], BF16, tag="sq")
            nc.vector.tensor_mul(t[:], s[:], cos_t[:, 0:T])
            sw = tmp.tile([128, T], BF16, tag="sq")
            for hb in range(2):
                b0 = hb * 64
                nc.scalar.copy(sw[b0:b0 + 32, :], s[b0 + 32:b0 + 64, :])
                nc.scalar.copy(sw[b0 + 32:b0 + 64, :], s[b0:b0 + 32, :])
            u = tmp.tile([128, T], BF16, tag="sq")
            nc.vector.tensor_mul(u[:], sw[:], sin_t[:, 0:T])
            nc.vector.tensor_add(s[:], t[:], u[:])

    wq_t = load_w_big("wq")
    wk_t = load_w_big("wk")
    wv_t = load_w_big("wv")
    qT = proj_cmajor(wq_t, xn, TQ, "bq", "qT")
    rope_inplace(qT, TQ)
    kT = proj_cmajor(wk_t, xn, TK, "bk", "kT")
    rope_inplace(kT, TK)
    v65 = proj_v65(wv_t, xn, bvr, "v65s")

    vn = layernorm_T([vt[:, cc * TK:(cc + 1) * TK] for cc in range(NCH)],
                     TK, "lnkv_w", "lnkv_nw", "lnkv_b")
    wcv_t = load_w_big("wcv")
    cv65 = proj_v65(wcv_t, vn, bcvr, "v65c")
    closepool(tabs_cm)
    wcq_t = load_w_big("wcq")
    wck_t = load_w_big("wck")

    # cq/ck are emitted incrementally: chunk j+1's matmuls are interleaved
    # into attention segment j as PE filler under the ACT-paced exp stream.
    cqT = [None] * NCH
    ckT = [None] * NCH

    def emit_cq(fc):
        o = qk_pool.tile([128, TQ], BF16, tag=f"cqT{fc}", name=f"cqT{fc}")
        ps = proj_ps.tile([128, 512], F32, tag="proj", name=f"cq_ps{fc}")
        for cc in range(NCH):
            nc.tensor.matmul(
                ps[:], wcq_t[:, cc * D + fc * 128:cc * D + (fc + 1) * 128],
                xn[cc][:, 0:TQ], start=(cc == 0), stop=(cc == NCH - 1))
        nc.vector.tensor_scalar_add(o[:], ps[:], pcol("bcq", fc))
        cqT[fc] = o

    def emit_ck_half(fc, h):
        if h == 0:
            ckT[fc] = qk_pool.tile([128, TK], BF16, tag=f"ckT{fc}",
                                   name=f"ckT{fc}")
        o = ckT[fc]
        cs = slice(h * 512, (h + 1) * 512)
        ps = proj_ps.tile([128, 512], F32, tag="proj", name=f"ck_ps{fc}_{h}")
        for cc in range(NCH):
            nc.tensor.matmul(
                ps[:], wck_t[:, cc * D + fc * 128:cc * D + (fc + 1) * 128],
                vn[cc][:, cs], start=(cc == 0), stop=(cc == NCH - 1))
        nc.vector.tensor_scalar_add(o[:, cs], ps[:], pcol("bck", fc))

    emit_cq(0)
    emit_ck_half(0, 0)
    emit_ck_half(0, 1)

    # ------------- attention -------------
    closepool(stat_cm)
    dp_cm, dpool = openpool(name="dpool", bufs=4, side="left")
    e_cm, e_pool = openpool(name="e", bufs=4, side="left")

    sc_cm, sc_ps = openpool(name="sc_ps", bufs=2, space="PSUM")
    avo_cm, avo_ps = openpool(name="avo_ps", bufs=2, space="PSUM")

    unnorm = []
    for j in range(NCH):              # head pair j: heads 2j, 2j+1
        ps_o = [avo_ps.tile([128, TQ], F32, tag="avo", name=f"avo{j}_{i}")
                for i in range(2)]

        def emit_av(e_tiles, kp):
            for i in range(2):
                h = 2 * j + i
                hsl = slice(h * VW, h * VW + DH + 1)
                for u in range(2):
                    kc = 2 * kp + u
                    v_src = v65[kc % 8] if kc < 8 else cv65[kc % 8]
                    nc.tensor.matmul(ps_o[i][0:DH + 1, :], v_src[:, hsl],
                                     e_tiles[i][:, u * 512:(u + 1) * 512],
                                     start=(kp == 0 and u == 0),
                                     stop=(kp == 7 and u == 1))

        pending = None
        for kp in range(8):           # pairs of key chunks
            e_tiles = []
            for i, p0 in enumerate((0, 64)):
                ps_s = sc_ps.tile([128, 1024], F32, tag="score",
                                  name=f"sc{j}_{kp}_{i}")
                for u in range(2):
                    kc = 2 * kp + u
                    k_src = kT[j] if kc < 8 else ckT[j]
                    q_src = qT[j] if kc < 8 else cqT[j]
                    csl = slice((kc % 8) * 128, (kc % 8) * 128 + 128)
                    nc.tensor.matmul(ps_s[:, u * 512:(u + 1) * 512],
                                     k_src[p0:p0 + 64, csl], q_src[p0:p0 + 64, :],
                                     start=True, stop=True, tile_position=(p0, 0))
                e = e_pool.tile([128, 1024], BF16, tag="e", name=f"e{j}_{kp}_{i}")
                nc.scalar.activation(e[:], ps_s[:], AF.Exp)
                e_tiles.append(e)
            # PE filler for the exp-wait gap: next chunk's cross projections
            if j < NCH - 1:
                if kp == 2:
                    emit_ck_half(j + 1, 0)
                elif kp == 4:
                    emit_ck_half(j + 1, 1)
                elif kp == 6:
                    emit_cq(j + 1)
            if pending is not None:
                emit_av(*pending)
            pending = (e_tiles, kp)
        emit_av(*pending)

        un = attn_p.tile([128, TQ], BF16, tag=f"attnT{j}", name=f"attnT{j}")
        nc.vector.tensor_copy(un[0:64, :], ps_o[0][0:DH, :])
        nc.vector.tensor_copy(un[64:128, :], ps_o[1][0:DH, :])
        d0 = dpool.tile([1, TQ], F32, tag="drow", name=f"d0_{j}")
        d1 = dpool.tile([1, TQ], F32, tag="drow", name=f"d1_{j}")
        nc.vector.tensor_copy(d0[:], ps_o[0][DH:DH + 1, :])
        nc.vector.tensor_copy(d1[:], ps_o[1][DH:DH + 1, :])
        r0 = dpool.tile([1, TQ], F32, tag="drow", name=f"r0_{j}")
        r1 = dpool.tile([1, TQ], F32, tag="drow", name=f"r1_{j}")
        nc.vector.reciprocal_approx_fast(r0[:], d0[:])
        nc.vector.reciprocal_approx_fast(r1[:], d1[:])
        ps_b = avo_ps.tile([128, TQ], F32, tag="avo", name=f"psb{j}")
        nc.tensor.matmul(ps_b[0:64, :], ones_row_f[:, 0:64], r0[:],
                         start=True, stop=True, tile_position=(0, 0))
        nc.tensor.matmul(ps_b[64:128, :], ones_row_f[:, 0:64], r1[:],
                         start=True, stop=True, tile_position=(0, 64))
        nc.vector.tensor_mul(un[:], un[:], ps_b[:])
        unnorm.append(un)

    closepool(avo_cm)
    closepool(sc_cm)
    closepool(proj_cm)
    attnT = unnorm
    closepool(e_cm)
    closepool(dp_cm)
    closepool(v65_cm)
    closepool(qk_cm)
    closepool(w_cm)
    closepool(inp_cm)        # frees xt/vt

    # wout + wf1 arrive during LNout/out-proj (wout DMA first: needed first)
    wf1_cm, wf1_p = openpool(name="wf1_p", bufs=1, side="right")
    wf1_a = wf1_p.tile([128, 4 * 4 * D], BF16)      # chunks cc 0-3
    wf1_b = wf1_p.tile([128, 4 * 4 * D], BF16)      # chunks cc 4-7
    wout_cm, wout_p = openpool(name="wout_p", bufs=1, side="right")
    wout_t = wout_p.tile([128, NCH * D], BF16)
    nc.sync.dma_start(out=wout_t[:].rearrange("p (c d) -> p c d", d=D),
                      in_=dram["wout"].rearrange("(c p) d -> p c d", p=128))
    nc.sync.dma_start(
        out=wf1_a[:].rearrange("p (c d) -> p c d", d=4 * D),
        in_=dram["wf1"][0:512, :].rearrange("(c p) d -> p c d", p=128))
    nc.sync.dma_start(
        out=wf1_b[:].rearrange("p (c d) -> p c d", d=4 * D),
        in_=dram["wf1"][512:1024, :].rearrange("(c p) d -> p c d", p=128))

    # ------------- LN + out projection + residual -------------
    proj_cm, proj_ps = openpool(name="proj_ps2", bufs=6, space="PSUM")
    stat_cm, stat_ps = openpool(name="ln_stat2", bufs=1, space="PSUM")

    zT = layernorm_T([a[:] for a in attnT], TQ, "lnout_w", "lnout_nw", "lnout_b")

    xnewT = []
    for fc in range(NCH):
        ps = proj_ps.tile([128, 512], F32, tag="proj")
        for cc in range(NCH):
            nc.tensor.matmul(ps[:], wout_t[:, cc * D + fc * 128:cc * D + (fc + 1) * 128],
                             zT[cc], start=(cc == 0), stop=(cc == NCH - 1))
        xnew = xnew_p.tile([128, TQ], BF16, tag=f"xnewT{fc}")
        nc.vector.scalar_tensor_tensor(xnew[:], ps[:], pcol("bout", fc),
                                       xown[:, fc * TQ:(fc + 1) * TQ],
                                       ALU.add, ALU.add)
        xnewT.append(xnew)
    closepool(xown_cm)
    closepool(attn_cm)
    closepool(wout_cm)

    # wf2 prefetch during FFN1 (right side, above wf1)
    wf2_cm, wf2_p = openpool(name="wf2_p", bufs=1, side="right")
    wf2_t = wf2_p.tile([128, 32 * D], BF16)
    nc.sync.dma_start(out=wf2_t[:].rearrange("p (c d) -> p c d", d=D),
                      in_=dram["wf2"].rearrange("(c p) d -> p c d", p=128))

    # ------------- FFN -------------
    xn3_cm, xn3_p = openpool(name="xn3", bufs=1, side="left")
    xn3 = [xn3_p.tile([128, TQ], BF16, tag=f"xn3_{cc}", name=f"xn3_{cc}")
           for cc in range(NCH)]
    layernorm_T([x[:] for x in xnewT], TQ, "lnffn_w", "lnffn_nw", "lnffn_b",
                dst_aps=[x[:] for x in xn3])

    h1_cm, h1_p = openpool(name="h1", bufs=1, side="left")
    h1 = []
    for fc in range(32):
        ps = proj_ps.tile([128, 512], F32, tag="proj")
        for cc in range(NCH):
            w_t = wf1_a if cc < 4 else wf1_b
            col = (cc % 4) * 4 * D + fc * 128
            nc.tensor.matmul(ps[:], w_t[:, col:col + 128], xn3[cc][:],
                             start=(cc == 0), stop=(cc == NCH - 1))
        o = h1_p.tile([128, TQ], BF16, tag=f"h1_{fc}")
        nc.scalar.activation(o[:], ps[:], AF.Gelu, bias=pcol("bf1", fc))
        h1.append(o)

    fin_cm, fin_p = openpool(name="fin", bufs=2, side="left")
    for fc in range(NCH):
        ps = proj_ps.tile([128, 512], F32, tag="proj")
        for cc in range(32):
            nc.tensor.matmul(ps[:], wf2_t[:, cc * D + fc * 128:cc * D + (fc + 1) * 128],
                             h1[cc][:], start=(cc == 0), stop=(cc == 31))
        fin = fin_p.tile([128, TQ], F32, tag="fin")
        nc.vector.scalar_tensor_tensor(fin[:], ps[:], pcol("bf2", fc),
                                       xnewT[fc][:], ALU.add, ALU.add)
        nc.sync.dma_start(out=dram["out"][fc * 128:(fc + 1) * 128, :], in_=fin[:])

    # ------------- teardown (LIFO per space/side) -------------
    closepool(stat_cm)
    closepool(proj_cm)
    closepool(wf2_cm)
    closepool(wf1_cm)
    closepool(fin_cm)
    closepool(h1_cm)
    closepool(xn3_cm)
    closepool(xnew_cm)
    closepool(lnsb_cm)
    closepool(rows1_cm)
    closepool(rows_cm)
    closepool(tmp_cm)
    closepool(const_cm)


def _prep_inputs(inputs):
    """Host-side sharding + weight preprocessing. Returns in_maps for 8 cores."""
    bf = ml_dtypes.bfloat16
    x = np.asarray(inputs["x"], np.float32)
    vggt = np.asarray(inputs["vggt"], np.float32)

    perm = np.concatenate([np.arange(0, DH, 2), np.arange(1, DH, 2)])
    scale = 1.0 / np.sqrt(DH)

    W_qkv = np.asarray(inputs["W_qkv"], np.float32).reshape(D, H, 3, DH)
    b_qkv = np.asarray(inputs["b_qkv"], np.float32).reshape(H, 3, DH)
    W_q = (W_qkv[:, :, 0, :][:, :, perm] * scale).reshape(D, D)
    b_q = (b_qkv[:, 0, :][:, perm] * scale).reshape(D)
    W_k = W_qkv[:, :, 1, :][:, :, perm].reshape(D, D)
    b_k = b_qkv[:, 1, :][:, perm].reshape(D)
    W_v = W_qkv[:, :, 2, :].reshape(D, D)
    b_v = b_qkv[:, 2, :].reshape(D)
    W_cq = np.asarray(inputs["W_cq"], np.float32) * scale
    b_cq = np.asarray(inputs["b_cq"], np.float32) * scale
    W_kv = np.asarray(inputs["W_kv"], np.float32).reshape(D, H, 2, DH)
    b_kv = np.asarray(inputs["b_kv"], np.float32).reshape(H, 2, DH)
    W_ck = W_kv[:, :, 0, :].reshape(D, D)
    b_ck = b_kv[:, 0, :].reshape(D)
    W_cv = W_kv[:, :, 1, :].reshape(D, D)
    b_cv = b_kv[:, 1, :].reshape(D)

    # rope tables in permuted space (64 rows), stacked x2 for 2-head tiles
    inv_freq = 1.0 / (10000.0 ** (np.arange(0, DH, 2, dtype=np.float32) / DH))
    t = np.arange(TK, dtype=np.float32)
    freqs = np.einsum("i,j->ij", t, inv_freq)
    emb = np.concatenate([freqs, freqs], axis=-1)
    cos, sin = np.cos(emb), np.sin(emb)
    cosP = np.ascontiguousarray(cos[:, perm].T).astype(np.float32)   # (64, T)
    sinP = np.empty((DH, TK), np.float32)
    sinP[0:32] = -sin[:, 0::2].T
    sinP[32:64] = +sin[:, 1::2].T

    def packcols(*vecs):
        cols = []
        for v in vecs:
            cols.append(np.asarray(v, np.float32).reshape(-1, 128).T)
        return np.ascontiguousarray(np.concatenate(cols, axis=1))

    ln = {k: np.asarray(inputs[k], np.float32) for k in
          ["ln_q_w", "ln_q_b", "ln_kv_w", "ln_kv_b", "ln_out_w", "ln_out_b",
           "ln_ffn_w", "ln_ffn_b"]}
    params = packcols(
        ln["ln_q_w"], -ln["ln_q_w"], ln["ln_q_b"],
        ln["ln_kv_w"], -ln["ln_kv_w"], ln["ln_kv_b"],
        ln["ln_out_w"], -ln["ln_out_w"], ln["ln_out_b"],
        ln["ln_ffn_w"], -ln["ln_ffn_w"], ln["ln_ffn_b"],
        b_q, b_k, b_cq, b_ck,
        np.asarray(inputs["b_out"], np.float32),
        np.asarray(inputs["b_f2"], np.float32),
        np.asarray(inputs["b_f1"], np.float32),
    )
    assert params.shape == (128, N_PARAM_COLS)

    common = {
        "wq": W_q.astype(bf), "wk": W_k.astype(bf), "wv": W_v.astype(bf),
        "wcq": W_cq.astype(bf), "wck": W_ck.astype(bf), "wcv": W_cv.astype(bf),
        "wout": np.asarray(inputs["W_out"], np.float32).astype(bf),
        "wf1": np.asarray(inputs["W_f1"], np.float32).astype(bf),
        "wf2": np.asarray(inputs["W_f2"], np.float32).astype(bf),
        "params": params,
        "bv_row": np.ascontiguousarray(b_v[None, :]).astype(bf),
        "bcv_row": np.ascontiguousarray(b_cv[None, :]).astype(bf),
    }

    in_maps = []
    for core in range(8):
        b, half = core // 2, core % 2
        if half == 0:
            order = np.arange(TK)
        else:
            order = np.concatenate([np.arange(TQ, TK), np.arange(0, TQ)])
        xl = x[b][order]
        m = dict(common)
        m["xT"] = np.ascontiguousarray(xl.T).astype(bf)
        m["vT"] = np.ascontiguousarray(vggt[b].T).astype(bf)
        ctab = cosP[:, order]
        stab = sinP[:, order]
        m["cosT"] = np.ascontiguousarray(np.concatenate([ctab, ctab], axis=0)).astype(bf)
        m["sinT"] = np.ascontiguousarray(np.concatenate([stab, stab], axis=0)).astype(bf)
        in_maps.append(m)
    return in_maps


def kernel(**inputs):
    trivial = all(np.all(np.asarray(inputs[k]) == 1.0) for k in
                  ["ln_q_w", "ln_kv_w", "ln_out_w", "ln_ffn_w"]) and \
              all(np.all(np.asarray(inputs[k]) == 0.0) for k in
                  ["ln_q_b", "ln_kv_b", "ln_out_b", "ln_ffn_b"])
    key = f"nc_{trivial}"
    if key not in _CACHE:
        _CACHE[key] = _build_program(trivial_ln=trivial)
    nc = _CACHE[key]
    in_maps = _prep_inputs(inputs)
    res = run_bass_kernel_spmd(nc, in_maps, list(range(8)),
                               **_CACHE.get("run_kwargs", {}))
    _CACHE["last_result"] = res
    outp = np.empty((4, TK, D), np.float32)
    for core in range(8):
        b, half = core // 2, core % 2
        outp[b, half * TQ:(half + 1) * TQ, :] = res.results[core]["out"].T
    return outp


# revision 21
# speedup vs baseline: 1.1192x; 1.0214x over previous
"""Trainium2 Bass kernel for nn_BridgeAttentionLayer (B=4, Tx=Tv=1024, D=1024, H=16).

Sharding: 8 cores = (batch b, query-token-half). Each core computes, for its
batch, the full K/V projections (self + cross) plus queries/attention/output
for its own 512 tokens. The host reorders tokens per core so "own" tokens are
always local positions 0:512 (attention is key-order invariant; RoPE tables
are passed per-core in matching order).

v2 (perf rewrite over the v1 baseline, 746us measured):
- Softmax normalization deferred out of the per-head-pair loop: AV outputs are
  copied unnormalized to SBUF, denominators collected into one [16,512] tile,
  one batched reciprocal_approx_fast + per-pair rank-1 broadcasts apply the
  normalization at the end.  (v1 ran a 3.3us single-lane RECIPROCAL on the
  critical path of every head pair, which kept the PE clock-gate cold.)
- exp batched to [128,1024] ACT instructions (2 key chunks per call).
- One DMA per tensor: weights/inputs laid out as [128, n*width] SBUF tiles via
  a (c p) d -> p (c d) rearrange of the DRAM source.
- LN squares + applies on DVE in bf16 (2x mode); broadcast rows go through
  the shared [128,512] proj PSUM pool and are copied once to SBUF bf16.
- cos/sin tables bf16; residual x comes from the bf16 xT tile (no fp32 xTo).
- Weight prefetch staged: attention-phase weights early (3 rotating slots),
  W_f1 + W_out during attention, W_f2 during FFN1.
"""

import numpy as np
import ml_dtypes

import concourse.bass as bass
import concourse.mybir as mybir
import concourse.tile as tile
from concourse import bacc
from concourse.bass_utils import run_bass_kernel_spmd

F32 = mybir.dt.float32
BF16 = mybir.dt.bfloat16
AF = mybir.ActivationFunctionType
ALU = mybir.AluOpType

D = 1024
H = 16
DH = 64
VW = 66           # per-head V tile width: 64 values + ones col + pad (4B align)
TQ = 512          # own query tokens per core
TK = 1024         # full sequence (keys)
NCH = 8           # D / 128
EPS = 1e-5

# packed per-partition param columns: name -> (start, n_chunks)
PARAM_COLS = {}
_off = 0
for _name, _n in [
    ("lnq_w", 8), ("lnq_nw", 8), ("lnq_b", 8),
    ("lnkv_w", 8), ("lnkv_nw", 8), ("lnkv_b", 8),
    ("lnout_w", 8), ("lnout_nw", 8), ("lnout_b", 8),
    ("lnffn_w", 8), ("lnffn_nw", 8), ("lnffn_b", 8),
    ("bq", 8), ("bk", 8), ("bcq", 8), ("bck", 8),
    ("bout", 8), ("bf2", 8), ("bf1", 32),
]:
    PARAM_COLS[_name] = (_off, _n)
    _off += _n
N_PARAM_COLS = _off

_CACHE = {}
_SIM_GELU_IDENTITY = False   # local CoreSim debugging only (sim lacks Gelu)


def _build_program(trivial_ln=False):
    nc = bacc.Bacc("TRN2", target_bir_lowering=False, debug=False, num_devices=8)

    def din(name, shape, dt):
        return nc.dram_tensor(name, shape, dt, kind="ExternalInput").ap()

    dram = {
        "xT": din("xT", [D, TK], BF16),        # x[b].T, local token order
        "vT": din("vT", [D, TK], BF16),        # vggt[b].T
        "wq": din("wq", [D, D], BF16),
        "wk": din("wk", [D, D], BF16),
        "wv": din("wv", [D, D], BF16),
        "wcq": din("wcq", [D, D], BF16),
        "wck": din("wck", [D, D], BF16),
        "wcv": din("wcv", [D, D], BF16),
        "wout": din("wout", [D, D], BF16),
        "wf1": din("wf1", [D, 4 * D], BF16),
        "wf2": din("wf2", [4 * D, D], BF16),
        "params": din("params", [128, N_PARAM_COLS], F32),
        "bv_row": din("bv_row", [1, D], BF16),
        "bcv_row": din("bcv_row", [1, D], BF16),
        "cosT": din("cosT", [128, TK], BF16),  # 2-head-stacked, permuted, local order
        "sinT": din("sinT", [128, TK], BF16),
        "out": nc.dram_tensor("out", [D, TQ], F32, kind="ExternalOutput").ap(),
    }

    with tile.TileContext(nc) as tc:
        _emit(nc, tc, dram, trivial_ln)

    nc.compile()
    return nc


def _emit(nc, tc, dram, trivial_ln):
    def openpool(**kw):
        cm = tc.tile_pool(**kw)
        return cm, cm.__enter__()

    def closepool(cm):
        cm.__exit__(None, None, None)

    # ------------- long-lived left-side SBUF pools -------------
    const_cm, const = openpool(name="const", bufs=1, side="left")
    pt = const.tile([128, N_PARAM_COLS], F32)
    nc.sync.dma_start(out=pt[:], in_=dram["params"][:])

    def pcol(name, i):
        start, n = PARAM_COLS[name]
        assert i < n
        return pt[:, start + i:start + i + 1]

    ones_col_bf = const.tile([128, 1], BF16)
    nc.any.memset(ones_col_bf[:], 1.0)
    ones_row_bf = const.tile([1, 128], BF16)
    nc.any.memset(ones_row_bf[:], 1.0)
    ones_row_f = const.tile([1, 128], F32)
    nc.any.memset(ones_row_f[:], 1.0)

    tmp_cm, tmp = openpool(name="tmp", bufs=3, side="left")      # sq/rope scratch
    rows_cm, rows = openpool(name="rows", bufs=3, side="left")   # srow scratch
    rows1_cm, rows1 = openpool(name="rows1", bufs=1, side="left")  # r/mr rows
    lnsb_cm, lnsb = openpool(name="lnsb", bufs=1, side="left")   # r/m bcast bf16
    xnew_cm, xnew_p = openpool(name="xnew", bufs=1, side="left")
    attn_cm, attn_p = openpool(name="attn", bufs=1, side="left")
    xown_cm, xown_p = openpool(name="xown", bufs=1, side="left")
    qk_cm, qk_pool = openpool(name="qk", bufs=1, side="left")
    v65_cm, v65_pool = openpool(name="v65", bufs=1, side="left")

    # ------------- right-side phase pools: inputs + proj weights -------------
    inp_cm, inp = openpool(name="inp", bufs=1, side="right")
    xt = inp.tile([128, NCH * TK], BF16)
    nc.sync.dma_start(out=xt[:].rearrange("p (c t) -> p c t", t=TK),
                      in_=dram["xT"].rearrange("(c p) t -> p c t", p=128))
    vt = inp.tile([128, NCH * TK], BF16)
    nc.sync.dma_start(out=vt[:].rearrange("p (c t) -> p c t", t=TK),
                      in_=dram["vT"].rearrange("(c p) t -> p c t", p=128))

    w_cm, wpool = openpool(name="w", bufs=2, side="right")
    tabs_cm, tabs = openpool(name="tabs", bufs=1, side="right")
    cos_t = tabs.tile([128, TK], BF16)
    nc.sync.dma_start(out=cos_t[:], in_=dram["cosT"][:])
    sin_t = tabs.tile([128, TK], BF16)
    nc.sync.dma_start(out=sin_t[:], in_=dram["sinT"][:])
    bvr = tabs.tile([1, D], BF16)
    nc.sync.dma_start(out=bvr[:], in_=dram["bv_row"][:])
    bcvr = tabs.tile([1, D], BF16)
    nc.sync.dma_start(out=bcvr[:], in_=dram["bcv_row"][:])

    def load_w_big(name):
        t = wpool.tile([128, NCH * D], BF16, tag="w")
        nc.sync.dma_start(
            out=t[:].rearrange("p (c d) -> p c d", d=D),
            in_=dram[name].rearrange("(c p) d -> p c d", p=128))
        return t

    # residual copy of own tokens (bf16) before LN overwrites xt
    xown = xown_p.tile([128, NCH * TQ], BF16)
    nc.vector.tensor_copy(
        xown[:].rearrange("p (c t) -> p c t", t=TQ),
        xt[:].rearrange("p (c t) -> p c t", t=TK)[:, :, 0:TQ])

    # ------------- PSUM pools for LN + projections -------------
    proj_cm, proj_ps = openpool(name="proj_ps", bufs=2, space="PSUM")
    stat_cm, stat_ps = openpool(name="ln_stat", bufs=1, space="PSUM")

    def layernorm_T(src_aps, T, wname, nwname, bname, dst_aps=None):
        """Transposed-space LN over NCH chunk APs [128, T] bf16 (in-place by
        default). Stats via ones-matmuls; rstd via Ln+Exp rows on ACT;
        broadcast through the [128,512] proj PSUM pool, copied once to SBUF
        bf16; apply on DVE in bf16 2x mode."""
        if dst_aps is None:
            dst_aps = src_aps
        nhalf = T // 512
        ps_s = [stat_ps.tile([1, 512], F32, tag=f"ps_s{h}", name=f"ps_s{h}")
                for h in range(nhalf)]
        ps_q = [stat_ps.tile([1, 512], F32, tag=f"ps_q{h}", name=f"ps_q{h}")
                for h in range(nhalf)]
        for cc in range(NCH):
            s = tmp.tile([128, T], BF16, tag="sq")
            nc.scalar.activation(s[:], src_aps[cc], AF.Square)
            for h in range(nhalf):
                cs = slice(h * 512, (h + 1) * 512)
                nc.tensor.matmul(ps_s[h][:], ones_col_bf[:], src_aps[cc][:, cs],
                                 start=(cc == 0), stop=(cc == NCH - 1))
                nc.tensor.matmul(ps_q[h][:], ones_col_bf[:], s[:, cs],
                                 start=(cc == 0), stop=(cc == NCH - 1))
        r_row = rows1.tile([1, T], F32, tag="r_row")
        mr_row = rows1.tile([1, T], F32, tag="mr_row")
        for h in range(nhalf):
            cs = slice(h * 512, (h + 1) * 512)
            m = rows.tile([1, 512], F32, tag="srow")
            nc.vector.tensor_scalar_mul(m[:], ps_s[h][:], 1.0 / D)
            msq = rows.tile([1, 512], F32, tag="srow")
            nc.vector.tensor_mul(msq[:], m[:], m[:])
            var = rows.tile([1, 512], F32, tag="srow")
            nc.vector.scalar_tensor_tensor(var[:], ps_q[h][:], 1.0 / D, msq[:],
                                           ALU.mult, ALU.subtract)
            nc.vector.tensor_scalar_add(var[:], var[:], EPS)
            # rstd = 1/Sqrt(var+eps): Sqrt on ACT (sqrt table set, no ln/exp
            # thrash), reciprocal via the fast approx on DVE
            sig = rows.tile([1, 512], F32, tag="srow")
            nc.scalar.activation(sig[:], var[:], AF.Sqrt)
            nc.vector.reciprocal_approx_fast(r_row[:, cs], sig[:])
            nc.vector.tensor_mul(mr_row[:, cs], m[:], r_row[:, cs])
        r_sb = lnsb.tile([128, T], BF16, tag="r_sb")
        m_sb = lnsb.tile([128, T], BF16, tag="m_sb")
        for h in range(nhalf):
            cs = slice(h * 512, (h + 1) * 512)
            ps_r = proj_ps.tile([128, 512], F32, tag="proj")
            nc.tensor.matmul(ps_r[:], ones_row_f[:], r_row[:, cs],
                             start=True, stop=True)
            nc.vector.tensor_copy(r_sb[:, cs], ps_r[:])
            ps_m = proj_ps.tile([128, 512], F32, tag="proj")
            nc.tensor.matmul(ps_m[:], ones_row_f[:], mr_row[:, cs],
                             start=True, stop=True)
            nc.vector.tensor_copy(m_sb[:, cs], ps_m[:])
        for cc in range(NCH):
            if trivial_ln:
                t1 = tmp.tile([128, T], BF16, tag="sq")
                nc.vector.tensor_mul(t1[:], src_aps[cc], r_sb[:])
                nc.vector.scalar_tensor_tensor(dst_aps[cc], m_sb[:], -1.0,
                                               t1[:], ALU.mult, ALU.add)
            else:
                t1 = tmp.tile([128, T], BF16, tag="sq")
                nc.vector.scalar_tensor_tensor(t1[:], src_aps[cc],
                                               pcol(wname, cc), r_sb[:],
                                               ALU.mult, ALU.mult)
                t2 = tmp.tile([128, T], BF16, tag="sq")
                nc.vector.scalar_tensor_tensor(t2[:], m_sb[:], pcol(nwname, cc),
                                               t1[:], ALU.mult, ALU.add)
                nc.vector.tensor_scalar_add(dst_aps[cc], t2[:], pcol(bname, cc))
        return dst_aps

    def proj_cmajor(w_t, rhs, T, bias_name, tag):
        """Y^T[fc] = sum_cc W[cc, fc-block].T @ rhs[cc][:, :T] -> NCH bf16 [128, T]."""
        outs = []
        for fc in range(NCH):
            o = qk_pool.tile([128, T], BF16, tag=f"{tag}{fc}")
            for h in range(T // 512):
                cs = slice(h * 512, (h + 1) * 512)
                ps = proj_ps.tile([128, 512], F32, tag="proj")
                for cc in range(NCH):
                    nc.tensor.matmul(
                        ps[:], w_t[:, cc * D + fc * 128:cc * D + (fc + 1) * 128],
                        rhs[cc][:, cs], start=(cc == 0), stop=(cc == NCH - 1))
                nc.vector.tensor_scalar_add(o[:, cs], ps[:], pcol(bias_name, fc))
            outs.append(o)
        return outs

    def proj_v65(w_t, rhs, bias_row, tag):
        """Token-major V with a ones column per head: NCH bf16 [128, H*VW]."""
        outs = []
        for tcb in range(NCH):
            o = v65_pool.tile([128, H * VW], BF16, tag=f"{tag}{tcb}")
            ones_view = o[:].rearrange("p (h w) -> p h w", w=VW)[:, :, DH:DH + 1]
            nc.vector.memset(ones_view, 1.0)
            for h in range(2):
                cs = slice(h * 512, (h + 1) * 512)
                ps = proj_ps.tile([128, 512], F32, tag="proj")
                for cc in range(NCH):
                    nc.tensor.matmul(ps[:], rhs[cc][:, tcb * 128:(tcb + 1) * 128],
                                     w_t[:, cc * D + h * 512:cc * D + (h + 1) * 512],
                                     start=(cc == 0), stop=False)
                nc.tensor.matmul(ps[:], ones_row_bf[:], bias_row[:, cs],
                                 start=False, stop=True)
                dst = o[:].rearrange("p (h w) -> p h w", w=VW)[:, h * 8:(h + 1) * 8, 0:DH]
                src = ps[:].rearrange("p (h w) -> p h w", w=DH)
                nc.vector.tensor_copy(dst, src)
            outs.append(o)
        return outs

    # ------------- LN(x); self projections + RoPE; LN(v); cross projections ---
    xn = layernorm_T([xt[:, cc * TK:(cc + 1) * TK] for cc in range(NCH)],
                     TK, "lnq_w", "lnq_nw", "lnq_b")

    def rope_inplace(tiles, T):
        for fc in range(NCH):
            s = tiles[fc]
            t = tmp.tile([128, T], BF16, tag="sq")
            nc.vector.tensor_mul(t[:], s[:], cos_t[:, 0:T])
            sw = tmp.tile([128, T], BF16, tag="sq")
            for hb in range(2):
                b0 = hb * 64
                nc.scalar.copy(sw[b0:b0 + 32, :], s[b0 + 32:b0 + 64, :])
                nc.scalar.copy(sw[b0 + 32:b0 + 64, :], s[b0:b0 + 32, :])
            u = tmp.tile([128, T], BF16, tag="sq")
            nc.vector.tensor_mul(u[:], sw[:], sin_t[:, 0:T])
            nc.vector.tensor_add(s[:], t[:], u[:])

    wq_t = load_w_big("wq")
    wk_t = load_w_big("wk")
    wv_t = load_w_big("wv")
    qT = proj_cmajor(wq_t, xn, TQ, "bq", "qT")
    vn = layernorm_T([vt[:, cc * TK:(cc + 1) * TK] for cc in range(NCH)],
                     TK, "lnkv_w", "lnkv_nw", "lnkv_b")
    rope_inplace(qT, TQ)
    kT = proj_cmajor(wk_t, xn, TK, "bk", "kT")
    rope_inplace(kT, TK)
    v65 = proj_v65(wv_t, xn, bvr, "v65s")
    wcv_t = load_w_big("wcv")
    cv65 = proj_v65(wcv_t, vn, bcvr, "v65c")
    closepool(tabs_cm)
    wcq_t = load_w_big("wcq")
    wck_t = load_w_big("wck")

    # cq/ck are emitted incrementally: chunk j+1's matmuls are interleaved
    # into attention segment j as PE filler under the ACT-paced exp stream.
    cqT = [None] * NCH
    ckT = [None] * NCH

    def emit_cq(fc):
        o = qk_pool.tile([128, TQ], BF16, tag=f"cqT{fc}", name=f"cqT{fc}")
        ps = proj_ps.tile([128, 512], F32, tag="proj", name=f"cq_ps{fc}")
        for cc in range(NCH):
            nc.tensor.matmul(
                ps[:], wcq_t[:, cc * D + fc * 128:cc * D + (fc + 1) * 128],
                xn[cc][:, 0:TQ], start=(cc == 0), stop=(cc == NCH - 1))
        nc.vector.tensor_scalar_add(o[:], ps[:], pcol("bcq", fc))
        cqT[fc] = o

    def emit_ck_half(fc, h):
        if h == 0:
            ckT[fc] = qk_pool.tile([128, TK], BF16, tag=f"ckT{fc}",
                                   name=f"ckT{fc}")
        o = ckT[fc]
        cs = slice(h * 512, (h + 1) * 512)
        ps = proj_ps.tile([128, 512], F32, tag="proj", name=f"ck_ps{fc}_{h}")
        for cc in range(NCH):
            nc.tensor.matmul(
                ps[:], wck_t[:, cc * D + fc * 128:cc * D + (fc + 1) * 128],
                vn[cc][:, cs], start=(cc == 0), stop=(cc == NCH - 1))
        nc.vector.tensor_scalar_add(o[:, cs], ps[:], pcol("bck", fc))

    emit_cq(0)
    emit_ck_half(0, 0)
    emit_ck_half(0, 1)

    # ------------- attention -------------
    closepool(stat_cm)
    dp_cm, dpool = openpool(name="dpool", bufs=3, side="left")
    e_cm, e_pool = openpool(name="e", bufs=3, side="left")

    sc_cm, sc_ps = openpool(name="sc_ps", bufs=2, space="PSUM")
    avo_cm, avo_ps = openpool(name="avo_ps", bufs=2, space="PSUM")

    unnorm = []
    pending_norm = [None]   # per-pair softmax normalization, emitted one
                            # segment late so its DVE chain never stalls PE
    for j in range(NCH):              # head pair j: heads 2j, 2j+1
        ps_o = [avo_ps.tile([128, TQ], F32, tag="avo", name=f"avo{j}_{i}")
                for i in range(2)]

        def emit_av(e_tiles, kp):
            for i in range(2):
                h = 2 * j + i
                hsl = slice(h * VW, h * VW + DH + 1)
                for u in range(2):
                    kc = 2 * kp + u
                    v_src = v65[kc % 8] if kc < 8 else cv65[kc % 8]
                    nc.tensor.matmul(ps_o[i][0:DH + 1, :], v_src[:, hsl],
                                     e_tiles[i][:, u * 512:(u + 1) * 512],
                                     start=(kp == 0 and u == 0),
                                     stop=(kp == 7 and u == 1))

        pending = None
        for kp in range(8):           # pairs of key chunks
            e_tiles = []
            for i, p0 in enumerate((0, 64)):
                ps_s = sc_ps.tile([128, 1024], F32, tag="score",
                                  name=f"sc{j}_{kp}_{i}")
                for u in range(2):
                    kc = 2 * kp + u
                    k_src = kT[j] if kc < 8 else ckT[j]
                    q_src = qT[j] if kc < 8 else cqT[j]
                    csl = slice((kc % 8) * 128, (kc % 8) * 128 + 128)
                    nc.tensor.matmul(ps_s[:, u * 512:(u + 1) * 512],
                                     k_src[p0:p0 + 64, csl], q_src[p0:p0 + 64, :],
                                     start=True, stop=True, tile_position=(p0, 0))
                e = e_pool.tile([128, 1024], BF16, tag="e", name=f"e{j}_{kp}_{i}")
                nc.scalar.activation(e[:], ps_s[:], AF.Exp)
                e_tiles.append(e)
            if kp == 1 and pending_norm[0] is not None:
                pending_norm[0]()
                pending_norm[0] = None
            # PE filler for the exp-wait gap: next chunk's cross projections
            if j < NCH - 1:
                if kp == 2:
                    emit_ck_half(j + 1, 0)
                elif kp == 4:
                    emit_ck_half(j + 1, 1)
                elif kp == 6:
                    emit_cq(j + 1)
            if pending is not None:
                emit_av(*pending)
            pending = (e_tiles, kp)
        emit_av(*pending)

        un = attn_p.tile([128, TQ], BF16, tag=f"attnT{j}", name=f"attnT{j}")
        nc.vector.tensor_copy(un[0:64, :], ps_o[0][0:DH, :])
        nc.vector.tensor_copy(un[64:128, :], ps_o[1][0:DH, :])
        d0 = dpool.tile([1, TQ], F32, tag="drow", name=f"d0_{j}")
        d1 = dpool.tile([1, TQ], F32, tag="drow", name=f"d1_{j}")
        nc.vector.tensor_copy(d0[:], ps_o[0][DH:DH + 1, :])
        nc.vector.tensor_copy(d1[:], ps_o[1][DH:DH + 1, :])
        r0 = dpool.tile([1, TQ], F32, tag="drow", name=f"r0_{j}")
        r1 = dpool.tile([1, TQ], F32, tag="drow", name=f"r1_{j}")
        nc.vector.reciprocal_approx_fast(r0[:], d0[:])
        nc.vector.reciprocal_approx_fast(r1[:], d1[:])

        def make_norm(j=j, un=un, r0=r0, r1=r1):
            def _norm():
                ps_b = proj_ps.tile([128, TQ], F32, tag="proj",
                                    name=f"psb{j}")
                nc.tensor.matmul(ps_b[0:64, :], ones_row_f[:, 0:64], r0[:],
                                 start=True, stop=True, tile_position=(0, 0))
                nc.tensor.matmul(ps_b[64:128, :], ones_row_f[:, 0:64], r1[:],
                                 start=True, stop=True, tile_position=(0, 64))
                nc.vector.tensor_mul(un[:], un[:], ps_b[:])
            return _norm

        pending_norm[0] = make_norm()
        unnorm.append(un)
    pending_norm[0]()

    closepool(avo_cm)
    closepool(sc_cm)
    closepool(proj_cm)
    attnT = unnorm
    closepool(e_cm)
    closepool(dp_cm)
    closepool(v65_cm)
    closepool(qk_cm)
    closepool(w_cm)
    closepool(inp_cm)        # frees xt/vt

    # wout + wf1 arrive during LNout/out-proj (wout DMA first: needed first)
    wf1_cm, wf1_p = openpool(name="wf1_p", bufs=1, side="right")
    wf1_a = wf1_p.tile([128, 4 * 4 * D], BF16)      # chunks cc 0-3
    wf1_b = wf1_p.tile([128, 4 * 4 * D], BF16)      # chunks cc 4-7
    wout_cm, wout_p = openpool(name="wout_p", bufs=1, side="right")
    wout_t = wout_p.tile([128, NCH * D], BF16)
    nc.sync.dma_start(out=wout_t[:].rearrange("p (c d) -> p c d", d=D),
                      in_=dram["wout"].rearrange("(c p) d -> p c d", p=128))
    nc.sync.dma_start(
        out=wf1_a[:].rearrange("p (c d) -> p c d", d=4 * D),
        in_=dram["wf1"][0:512, :].rearrange("(c p) d -> p c d", p=128))
    nc.sync.dma_start(
        out=wf1_b[:].rearrange("p (c d) -> p c d", d=4 * D),
        in_=dram["wf1"][512:1024, :].rearrange("(c p) d -> p c d", p=128))

    # ------------- LN + out projection + residual -------------
    proj_cm, proj_ps = openpool(name="proj_ps2", bufs=6, space="PSUM")
    stat_cm, stat_ps = openpool(name="ln_stat2", bufs=1, space="PSUM")

    zT = layernorm_T([a[:] for a in attnT], TQ, "lnout_w", "lnout_nw", "lnout_b")

    xnewT = []
    for fc in range(NCH):
        ps = proj_ps.tile([128, 512], F32, tag="proj")
        for cc in range(NCH):
            nc.tensor.matmul(ps[:], wout_t[:, cc * D + fc * 128:cc * D + (fc + 1) * 128],
                             zT[cc], start=(cc == 0), stop=(cc == NCH - 1))
        xnew = xnew_p.tile([128, TQ], BF16, tag=f"xnewT{fc}")
        nc.vector.scalar_tensor_tensor(xnew[:], ps[:], pcol("bout", fc),
                                       xown[:, fc * TQ:(fc + 1) * TQ],
                                       ALU.add, ALU.add)
        xnewT.append(xnew)
    closepool(xown_cm)
    closepool(attn_cm)
    closepool(wout_cm)

    # wf2 prefetch during FFN1 (right side, above wf1)
    wf2_cm, wf2_p = openpool(name="wf2_p", bufs=1, side="right")
    wf2_t = wf2_p.tile([128, 32 * D], BF16)
    nc.sync.dma_start(out=wf2_t[:].rearrange("p (c d) -> p c d", d=D),
                      in_=dram["wf2"].rearrange("(c p) d -> p c d", p=128))

    # ------------- FFN -------------
    xn3_cm, xn3_p = openpool(name="xn3", bufs=1, side="left")
    xn3 = [xn3_p.tile([128, TQ], BF16, tag=f"xn3_{cc}", name=f"xn3_{cc}")
           for cc in range(NCH)]
    layernorm_T([x[:] for x in xnewT], TQ, "lnffn_w", "lnffn_nw", "lnffn_b",
                dst_aps=[x[:] for x in xn3])

    h1_cm, h1_p = openpool(name="h1", bufs=1, side="left")
    h1 = []
    for fc in range(32):
        ps = proj_ps.tile([128, 512], F32, tag="proj")
        for cc in range(NCH):
            w_t = wf1_a if cc < 4 else wf1_b
            col = (cc % 4) * 4 * D + fc * 128
            nc.tensor.matmul(ps[:], w_t[:, col:col + 128], xn3[cc][:],
                             start=(cc == 0), stop=(cc == NCH - 1))
        o = h1_p.tile([128, TQ], BF16, tag=f"h1_{fc}")
        gf = AF.Identity if _SIM_GELU_IDENTITY else AF.Gelu
        nc.scalar.activation(o[:], ps[:], gf, bias=pcol("bf1", fc))
        h1.append(o)

    fin_cm, fin_p = openpool(name="fin", bufs=2, side="left")
    for fc in range(NCH):
        ps = proj_ps.tile([128, 512], F32, tag="proj")
        for cc in range(32):
            nc.tensor.matmul(ps[:], wf2_t[:, cc * D + fc * 128:cc * D + (fc + 1) * 128],
                             h1[cc][:], start=(cc == 0), stop=(cc == 31))
        fin = fin_p.tile([128, TQ], F32, tag="fin")
        nc.vector.scalar_tensor_tensor(fin[:], ps[:], pcol("bf2", fc),
                                       xnewT[fc][:], ALU.add, ALU.add)
        nc.sync.dma_start(out=dram["out"][fc * 128:(fc + 1) * 128, :], in_=fin[:])

    # ------------- teardown (LIFO per space/side) -------------
    closepool(stat_cm)
    closepool(proj_cm)
    closepool(wf2_cm)
    closepool(wf1_cm)
    closepool(fin_cm)
    closepool(h1_cm)
    closepool(xn3_cm)
    closepool(xnew_cm)
    closepool(lnsb_cm)
    closepool(rows1_cm)
    closepool(rows_cm)
    closepool(tmp_cm)
    closepool(const_cm)


def _prep_inputs(inputs):
    """Host-side sharding + weight preprocessing. Returns in_maps for 8 cores."""
    bf = ml_dtypes.bfloat16
    x = np.asarray(inputs["x"], np.float32)
    vggt = np.asarray(inputs["vggt"], np.float32)

    perm = np.concatenate([np.arange(0, DH, 2), np.arange(1, DH, 2)])
    scale = 1.0 / np.sqrt(DH)

    W_qkv = np.asarray(inputs["W_qkv"], np.float32).reshape(D, H, 3, DH)
    b_qkv = np.asarray(inputs["b_qkv"], np.float32).reshape(H, 3, DH)
    W_q = (W_qkv[:, :, 0, :][:, :, perm] * scale).reshape(D, D)
    b_q = (b_qkv[:, 0, :][:, perm] * scale).reshape(D)
    W_k = W_qkv[:, :, 1, :][:, :, perm].reshape(D, D)
    b_k = b_qkv[:, 1, :][:, perm].reshape(D)
    W_v = W_qkv[:, :, 2, :].reshape(D, D)
    b_v = b_qkv[:, 2, :].reshape(D)
    W_cq = np.asarray(inputs["W_cq"], np.float32) * scale
    b_cq = np.asarray(inputs["b_cq"], np.float32) * scale
    W_kv = np.asarray(inputs["W_kv"], np.float32).reshape(D, H, 2, DH)
    b_kv = np.asarray(inputs["b_kv"], np.float32).reshape(H, 2, DH)
    W_ck = W_kv[:, :, 0, :].reshape(D, D)
    b_ck = b_kv[:, 0, :].reshape(D)
    W_cv = W_kv[:, :, 1, :].reshape(D, D)
    b_cv = b_kv[:, 1, :].reshape(D)

    # rope tables in permuted space (64 rows), stacked x2 for 2-head tiles
    inv_freq = 1.0 / (10000.0 ** (np.arange(0, DH, 2, dtype=np.float32) / DH))
    t = np.arange(TK, dtype=np.float32)
    freqs = np.einsum("i,j->ij", t, inv_freq)
    emb = np.concatenate([freqs, freqs], axis=-1)
    cos, sin = np.cos(emb), np.sin(emb)
    cosP = np.ascontiguousarray(cos[:, perm].T).astype(np.float32)   # (64, T)
    sinP = np.empty((DH, TK), np.float32)
    sinP[0:32] = -sin[:, 0::2].T
    sinP[32:64] = +sin[:, 1::2].T

    def packcols(*vecs):
        cols = []
        for v in vecs:
            cols.append(np.asarray(v, np.float32).reshape(-1, 128).T)
        return np.ascontiguousarray(np.concatenate(cols, axis=1))

    ln = {k: np.asarray(inputs[k], np.float32) for k in
          ["ln_q_w", "ln_q_b", "ln_kv_w", "ln_kv_b", "ln_out_w", "ln_out_b",
           "ln_ffn_w", "ln_ffn_b"]}
    params = packcols(
        ln["ln_q_w"], -ln["ln_q_w"], ln["ln_q_b"],
        ln["ln_kv_w"], -ln["ln_kv_w"], ln["ln_kv_b"],
        ln["ln_out_w"], -ln["ln_out_w"], ln["ln_out_b"],
        ln["ln_ffn_w"], -ln["ln_ffn_w"], ln["ln_ffn_b"],
        b_q, b_k, b_cq, b_ck,
        np.asarray(inputs["b_out"], np.float32),
        np.asarray(inputs["b_f2"], np.float32),
        np.asarray(inputs["b_f1"], np.float32),
    )
    assert params.shape == (128, N_PARAM_COLS)

    common = {
        "wq": W_q.astype(bf), "wk": W_k.astype(bf), "wv": W_v.astype(bf),
        "wcq": W_cq.astype(bf), "wck": W_ck.astype(bf), "wcv": W_cv.astype(bf),
        "wout": np.asarray(inputs["W_out"], np.float32).astype(bf),
        "wf1": np.asarray(inputs["W_f1"], np.float32).astype(bf),
        "wf2": np.asarray(inputs["W_f2"], np.float32).astype(bf),
        "params": params,
        "bv_row": np.ascontiguousarray(b_v[None, :]).astype(bf),
        "bcv_row": np.ascontiguousarray(b_cv[None, :]).astype(bf),
    }

    in_maps = []
    for core in range(8):
        b, half = core // 2, core % 2
        if half == 0:
            order = np.arange(TK)
        else:
            order = np.concatenate([np.arange(TQ, TK), np.arange(0, TQ)])
        xl = x[b][order]
        m = dict(common)
        m["xT"] = np.ascontiguousarray(xl.T).astype(bf)
        m["vT"] = np.ascontiguousarray(vggt[b].T).astype(bf)
        ctab = cosP[:, order]
        stab = sinP[:, order]
        m["cosT"] = np.ascontiguousarray(np.concatenate([ctab, ctab], axis=0)).astype(bf)
        m["sinT"] = np.ascontiguousarray(np.concatenate([stab, stab], axis=0)).astype(bf)
        in_maps.append(m)
    return in_maps


def kernel(**inputs):
    trivial = all(np.all(np.asarray(inputs[k]) == 1.0) for k in
                  ["ln_q_w", "ln_kv_w", "ln_out_w", "ln_ffn_w"]) and \
              all(np.all(np.asarray(inputs[k]) == 0.0) for k in
                  ["ln_q_b", "ln_kv_b", "ln_out_b", "ln_ffn_b"])
    key = f"nc_{trivial}"
    if key not in _CACHE:
        _CACHE[key] = _build_program(trivial_ln=trivial)
    nc = _CACHE[key]
    in_maps = _prep_inputs(inputs)
    res = run_bass_kernel_spmd(nc, in_maps, list(range(8)),
                               **_CACHE.get("run_kwargs", {}))
    _CACHE["last_result"] = res
    outp = np.empty((4, TK, D), np.float32)
    for core in range(8):
        b, half = core // 2, core % 2
        outp[b, half * TQ:(half + 1) * TQ, :] = res.results[core]["out"].T
    return outp
